# revision 1
# baseline (speedup 1.0000x reference)
"""Trainium2 Bass kernel for causal multi-head attention + output projection.

Problem: B=4, T=2048, C=1024, H=16 heads (hd=64), causal softmax with
scale C**-0.5, then nn.Linear(C, C): y = attn_out @ W_out.T + b_out.

Sharding (8 cores): core = (b, head_half); b = core // 2, half = core % 2.
Each core computes 8 heads (4 head-pairs) over ALL 2048 rows of its batch
element -- every core runs the identical SPMD program (the causal loop
structure does not depend on the core id; only the data differs).  The
output projection contracts only the core's 512 channels, producing a
partial sum; the host adds the two partials per batch (replacing the
all-reduce) and adds the bias.

On-chip layout notes:
 - scoresT orientation: scores^T[k, q] = kT.T @ qT per head, so softmax
   denominators come from a ones-column appended to V (attn@V computes
   [65, q]: rows 0..63 = head dims, row 64 = sum of exp).
 - q/k/W are pre-transposed on the host (bf16), so no on-chip transposes.
 - Head pairs run as concurrent K=64 row-tiled matmuls (partitions 0-63 /
   64-127 of the PE array).
 - exp runs on ACT from PSUM with scale=C**-0.5 folded in; causal masking
   multiplies a precomputed 128x128 staircase on the diagonal blocks only.
"""

import os
import sys

for _p in ("/opt/trn_rl_repo", "/root/.axon_site/_ro/trn_rl_repo"):
    if os.path.isdir(_p) and _p not in sys.path:
        sys.path.insert(0, _p)

import numpy as np
import ml_dtypes
from contextlib import ExitStack

bf16 = ml_dtypes.bfloat16

B, T, C, H, HD = 4, 2048, 1024, 16, 64
NCORES = 8
HPC = H // 2          # heads per core = 8
PAIRS = HPC // 2      # head pairs per core = 4
CH = C // 2           # channels per core = 512
SCALE = float(C) ** -0.5

_CACHED = {}

# Tuning knobs (overridable before _get_program() for sim sweeps)
TUNE = {
    "at_bufs": 6,       # attnT SBUF pool bufs
    "sc_bufs": 2,       # scores PSUM pool bufs (tiles of 2 banks each)
    "ac_bufs": 3,       # accumulator PSUM pool bufs (1 bank each)
    "pj_bufs": 1,       # projection PSUM pool bufs (1 bank each)
    "y_bufs": 3,
    "rb_psum_mult": True,
    "pipe_depth": 2,    # tiles attn@V trails the scores/exp stream by
    "nb_delay": 4,      # tiles between a pair's reciprocals and its norm_b
    "pp_every": 4,      # emit a deferred projection chunk every N tiles
    "pj_pool": True,
    "flush_ge": 1,
    "loop_n": 1,        # >1: repeat body in one NEFF (benchmark mode)
    "level": 4,         # build truncation for phase benchmarks (4=full)
    "proj_interleave": True,
    "proj_dma": True,
    "proj_src_const": False,
    "y_on_act": False,  # projection PSUM->SBUF copies on ACT instead of DVE
}


def _patch_act_tables():
    # The act-table placement pass maps each ACT func to the lowest-id set
    # containing it, which thrashes between exp_and_others and natural_log
    # (2.7us per reload).  Strip Exp/Ln/Copy from every set except the
    # combined natural_log_exp_and_others so all our ACT work shares one
    # table load.  Set ids (dict order) are preserved.
    from concourse import bacc as _bacc
    from concourse.hw_specs import get_activation_tables as _orig

    if getattr(_bacc, "_mha_act_patch", False):
        return
    import concourse.mybir as mybir

    keep = {
        mybir.ActivationFunctionType.Exp,
        mybir.ActivationFunctionType.Ln,
        mybir.ActivationFunctionType.Copy,
    }

    def patched(arch):
        tables = {k: set(v) for k, v in _orig(arch).items()}
        if "natural_log_exp_and_others" in tables and keep <= tables[
            "natural_log_exp_and_others"
        ]:
            for name, fns in tables.items():
                if name != "natural_log_exp_and_others":
                    fns -= keep
        return tables

    _bacc.get_activation_tables = patched
    _bacc._mha_act_patch = True


def _build_program():
    import concourse.bass as bass
    import concourse.tile as tile
    import concourse.mybir as mybir
    from concourse import bacc

    _patch_act_tables()

    f32 = mybir.dt.float32
    bf = mybir.dt.bfloat16
    Exp = mybir.ActivationFunctionType.Exp
    Ln = mybir.ActivationFunctionType.Ln

    nc = bacc.Bacc()
    qT_d = nc.declare_dram_parameter("qT", [CH, T], bf, isOutput=False)
    kT_d = nc.declare_dram_parameter("kT", [CH, T], bf, isOutput=False)
    vx_d = nc.declare_dram_parameter("vx", [T, HPC * 65], bf, isOutput=False)
    wT_d = nc.declare_dram_parameter("wT", [CH, C], bf, isOutput=False)
    mk_d = nc.declare_dram_parameter("mk", [128, 256], bf, isOutput=False)
    on_d = nc.declare_dram_parameter("on", [1, 64], bf, isOutput=False)
    yp_d = nc.declare_dram_parameter("yp", [T, C], bf, isOutput=True)

    with tile.TileContext(nc) as tc, ExitStack() as ctx:
        const = ctx.enter_context(tc.tile_pool(name="const", bufs=1))

        # Persistent SBUF tensors
        qT_sb = const.tile([128, PAIRS, T], bf)      # [p, j, t]; p = pair channel
        kT_sb = const.tile([128, PAIRS, T], bf)
        v_sb = const.tile([128, 16, HPC * 65], bf)   # [p, kb, h*65+e]
        wt_sb = const.tile([128, PAIRS, C], bf)      # [p, ci, n]
        mk_sb = const.tile([128, 256], bf)
        on_sb = const.tile([128, 64], bf)            # row 64 holds ones
        aoT_sb = const.tile([128, PAIRS, T], bf)     # attn outputs, transposed

        for j in range(PAIRS):
            nc.sync.dma_start(
                qT_sb[:, j, :],
                qT_d[:].rearrange("(j p) t -> j p t", p=128)[j],
            )
            nc.sync.dma_start(
                kT_sb[:, j, :],
                kT_d[:].rearrange("(j p) t -> j p t", p=128)[j],
            )
            nc.sync.dma_start(
                wt_sb[:, j, :],
                wT_d[:].rearrange("(j p) n -> j p n", p=128)[j],
            )
        vx_r = vx_d[:].rearrange("(g kb p) e -> g p kb e", p=128, g=4)
        for g in range(4):
            nc.sync.dma_start(v_sb[:, 4 * g : 4 * (g + 1), :], vx_r[g])
        nc.sync.dma_start(mk_sb[:], mk_d[:])
        nc.sync.dma_start(on_sb[64:65, :], on_d[:])

        mk_v = mk_sb[:].rearrange("p (g o) -> p g o", g=2)

        atp = ctx.enter_context(tc.tile_pool(name="attnT", bufs=TUNE["at_bufs"]))
        sums = ctx.enter_context(tc.tile_pool(name="sums", bufs=3))
        rbsp = ctx.enter_context(tc.tile_pool(name="rbs", bufs=3))
        tmpp = ctx.enter_context(tc.tile_pool(name="tmp", bufs=3))
        ypool = ctx.enter_context(tc.tile_pool(name="yout", bufs=TUNE["y_bufs"]))
        ps_sc = ctx.enter_context(
            tc.tile_pool(name="ps_sc", bufs=TUNE["sc_bufs"], space="PSUM")
        )
        ps_ac = ctx.enter_context(
            tc.tile_pool(name="ps_ac", bufs=TUNE["ac_bufs"], space="PSUM")
        )
        ps_pj = (
            ctx.enter_context(
                tc.tile_pool(name="ps_pj", bufs=TUNE["pj_bufs"], space="PSUM")
            )
            if TUNE["pj_pool"]
            else None
        )

        def norm_b(p):
            # deferred normalize: recips via exp(-ln), broadcast across
            # partitions (K=1 matmuls), scale the SBUF copies into aoT_sb
            jj, qq0, un, r32, pid = p
            with nc.named_scope("norm"):
                rbf = sums.tile([65, 1024], bf, tag="rbf", name=f"rbf_{pid}")
                nc.scalar.activation(
                    out=rbf[64:65, :], in_=r32[64:65, :], func=Exp, scale=-1.0
                )
                rb = ps_sc.tile([128, 2, 512], f32, tag="sc", name=f"rb_{pid}")
                for h in range(2):
                    nc.tensor.matmul(
                        out=rb[0:64, h, :],
                        lhsT=on_sb[64:65, :],
                        rhs=rbf[64:65, h * 512 : (h + 1) * 512],
                        start=True,
                        stop=True,
                    )
                if TUNE["rb_psum_mult"]:
                    # un is in SBUF, so the multiplies can read the
                    # broadcast directly from PSUM (single-PSUM-operand op)
                    rb0 = rb[0:64, 0, :]
                    rb1 = rb[0:64, 1, :]
                else:
                    rb_sb = rbsp.tile([64, 1024], bf, tag="rbsb", name=f"rbsb_{pid}")
                    nc.vector.tensor_copy(rb_sb[:, :], rb[0:64, :, :])
                    rb0 = rb_sb[0:64, 0:512]
                    rb1 = rb_sb[0:64, 512:1024]
                nc.vector.tensor_mul(
                    out=aoT_sb[0:64, jj, qq0 : qq0 + 512],
                    in0=un[:, 0:512],
                    in1=rb0,
                )
                t1 = tmpp.tile([64, 512], bf, tag="t1", name=f"t1_{pid}")
                nc.vector.tensor_mul(out=t1[:, :], in0=un[:, 512:1024], in1=rb1)
                nc.sync.dma_start(aoT_sb[64:128, jj, qq0 : qq0 + 512], t1[:, :])

        def emit_proj(qc):
            # partial projection (contract the core's CH channels) for one
            # 128-row chunk
            if LEVEL < 4:
                return
            src_t = wt_sb if TUNE["proj_src_const"] else aoT_sb
            with nc.named_scope("proj"):
                y_sb = ypool.tile([128, C], bf, tag="y", name=f"y_{qc}")
                pjt = (
                    None
                    if ps_pj is not None
                    else ps_sc.tile([128, 2, 512], f32, tag="sc", name=f"pj_{qc}")
                )
                for nt in range(2):
                    if pjt is None:
                        pj = ps_pj.tile([128, 512], f32, tag="pj", name=f"pj_{qc}_{nt}")
                    else:
                        pj = pjt[:, nt, :]
                    for ci in range(PAIRS):
                        nc.tensor.matmul(
                            out=pj[:, :],
                            lhsT=src_t[:, ci, qc * 128 % src_t.shape[2] : qc * 128 % src_t.shape[2] + 128],
                            rhs=wt_sb[:, ci, nt * 512 : (nt + 1) * 512],
                            start=(ci == 0),
                            stop=(ci == PAIRS - 1),
                        )
                    if TUNE["y_on_act"]:
                        nc.scalar.copy(y_sb[:, nt * 512 : (nt + 1) * 512], pj[:, :])
                    else:
                        nc.vector.tensor_copy(
                            y_sb[:, nt * 512 : (nt + 1) * 512], pj[:, :]
                        )
                if TUNE["proj_dma"]:
                    nc.sync.dma_start(yp_d[qc * 128 : (qc + 1) * 128, :], y_sb[:, :])

        pending_proj = []

        # Optional benchmark loop: repeat the whole compute body loop_n
        # times in one NEFF (the body is idempotent) to measure HW time as
        # a wall-clock delta without per-call transfer overhead.
        loop_ctx = (
            tc.For_i(0, TUNE["loop_n"], 1) if TUNE["loop_n"] > 1 else None
        )
        if loop_ctx is not None:
            ctx.enter_context(loop_ctx)

        # Flat software-pipelined stream over (pair, kb): scores+exp lead,
        # attn@V trails by pipe_depth tiles (crossing pair boundaries), the
        # normalize broadcast trails one pair, projection trails one q-tile.
        DEPTH = TUNE["pipe_depth"]
        LEVEL = TUNE["level"]
        NB_DELAY = TUNE["nb_delay"]
        PP_EVERY = TUNE["pp_every"]
        normed = []  # (tick, norm_b payload) awaiting emission
        tick = [0]
        for qt in range(4):
            q0 = qt * 512
            nkb = 4 * qt + 4
            accs = {}
            pends = []       # (j, kb, c0, at)
            done_pairs = []  # pairs whose last attn@V has been emitted

            def pop_av():
                pj_, pkb, pc0, pat = pends.pop(0)
                if pkb == 0:
                    # free the oldest pair's acc banks before a new pair's
                    # accumulation claims slots (deadlock avoidance): with 4
                    # acc bufs, two pairs may be in flight, so only the
                    # second-oldest pending normalize must flush here
                    while len(normed) >= TUNE["flush_ge"]:
                        norm_b(normed.pop(0)[1])
                pacc = accs[pj_]
                if LEVEL >= 2:
                    with nc.named_scope("av"):
                        for h in range(2):
                            nc.tensor.matmul(
                                out=pacc[h][0:65, pc0:],
                                lhsT=v_sb[
                                    :, pkb, (2 * pj_ + h) * 65 : (2 * pj_ + h) * 65 + 65
                                ],
                                rhs=pat[:, h, pc0:],
                                start=(pkb == 0),
                                stop=(pkb == nkb - 1),
                            )
                if pkb == nkb - 1:
                    done_pairs.append(pj_)

            def flush_done():
                # norm_a (reciprocals) for any pair whose attn@V finished
                while done_pairs:
                    dj = done_pairs.pop(0)
                    dacc = accs.pop(dj)
                    if LEVEL < 3:
                        continue
                    # Release the acc PSUM banks fast: ln of the sum rows
                    # on ACT (1/s computed later as exp(-ln s); the DVE
                    # reciprocal is an iterative divide, ~8x the cost) plus
                    # DVE copies of the unnormalized outputs to SBUF.
                    with nc.named_scope("norm"):
                        r32 = sums.tile(
                            [65, 1024], f32, tag="r32", name=f"r32_{qt}_{dj}"
                        )
                        nc.scalar.activation(
                            out=r32[64:65, 0:512], in_=dacc[0][64:65, :], func=Ln
                        )
                        nc.scalar.activation(
                            out=r32[64:65, 512:1024], in_=dacc[1][64:65, :], func=Ln
                        )
                        un = sums.tile([64, 1024], bf, tag="un", name=f"un_{qt}_{dj}")
                        nc.vector.tensor_copy(un[:, 0:512], dacc[0][0:64, :])
                        nc.vector.tensor_copy(un[:, 512:1024], dacc[1][0:64, :])
                    normed.append((tick[0], (dj, q0, un, r32, f"{qt}_{dj}")))

            for j in range(PAIRS):
                accs[j] = [
                    ps_ac.tile([128, 512], f32, tag="acc", name=f"acc0_{qt}_{j}"),
                    ps_ac.tile([128, 512], f32, tag="acc", name=f"acc1_{qt}_{j}"),
                ]
                for kb in range(nkb):
                    c0 = max(0, (kb - 4 * qt) * 128)
                    sc = ps_sc.tile(
                        [128, 2, 512], f32, tag="sc", name=f"sc_{qt}_{j}_{kb}"
                    )
                    with nc.named_scope("sc"):
                        for h in range(2):
                            nc.tensor.matmul(
                                out=sc[:, h, c0:],
                                lhsT=kT_sb[
                                    h * 64 : (h + 1) * 64, j, kb * 128 : (kb + 1) * 128
                                ],
                                rhs=qT_sb[h * 64 : (h + 1) * 64, j, q0 + c0 : q0 + 512],
                                start=True,
                                stop=True,
                            )
                    at = atp.tile([128, 2, 512], bf, tag="at", name=f"at_{qt}_{j}_{kb}")
                    if LEVEL >= 1:
                        with nc.named_scope("exp"):
                            nc.scalar.activation(
                                out=at[:, :, c0:],
                                in_=sc[:, :, c0:],
                                func=Exp,
                                scale=SCALE,
                            )
                        if kb >= 4 * qt:  # diagonal block: causal staircase mask
                            with nc.named_scope("mask"):
                                nc.vector.tensor_mul(
                                    out=at[:, :, c0 : c0 + 128],
                                    in0=at[:, :, c0 : c0 + 128],
                                    in1=mk_v,
                                )
                    pends.append((j, kb, c0, at))
                    if len(pends) > DEPTH:
                        pop_av()
                        flush_done()
                    tick[0] += 1
                    # interleave deferred work from previous pairs/q-tiles
                    if normed and tick[0] - normed[0][0] >= NB_DELAY:
                        norm_b(normed.pop(0)[1])
                    if (
                        TUNE["proj_interleave"]
                        and pending_proj
                        and tick[0] % PP_EVERY == 0
                        and qt > 0
                    ):
                        emit_proj(pending_proj.pop(0))

            while pends:
                pop_av()
            flush_done()
            if qt == 3:
                while normed:
                    norm_b(normed.pop(0)[1])
            pending_proj.extend(range(4 * qt, 4 * qt + 4))
            if qt == 3:
                while pending_proj:
                    emit_proj(pending_proj.pop(0))

    nc.finalize()
    return nc


def _get_program():
    if "nc" not in _CACHED:
        _CACHED["nc"] = _build_program()
    return _CACHED["nc"]


def _prep_inputs(q, k, v, W_out):
    """Build the 8 per-core input maps (host-side shard + transpose + cast)."""
    mk = np.zeros((128, 2, 128), np.float32)
    kk = np.arange(128)[:, None]
    oo = np.arange(128)[None, :]
    mk[:, 0, :] = (kk <= oo).astype(np.float32)
    mk[:, 1, :] = mk[:, 0, :]
    mk = mk.reshape(128, 256).astype(bf16)
    ones = np.ones((1, 64), bf16)

    in_maps = []
    for core in range(NCORES):
        b, hh = core // 2, core % 2
        ch0 = hh * CH
        qT = np.ascontiguousarray(q[b].T[ch0 : ch0 + CH]).astype(bf16)
        kT = np.ascontiguousarray(k[b].T[ch0 : ch0 + CH]).astype(bf16)
        vh = v[b].reshape(T, H, HD)[:, hh * HPC : (hh + 1) * HPC, :]
        vx = np.concatenate(
            [vh.astype(np.float32), np.ones((T, HPC, 1), np.float32)], axis=2
        )
        vx = np.ascontiguousarray(vx.reshape(T, HPC * 65)).astype(bf16)
        wT = np.ascontiguousarray(W_out.T[ch0 : ch0 + CH]).astype(bf16)
        in_maps.append(
            {"qT": qT, "kT": kT, "vx": vx, "wT": wT, "mk": mk, "on": ones}
        )
    return in_maps


def _run(in_maps, trace=False):
    from concourse.bass_utils import run_bass_kernel_spmd

    nc = _get_program()
    return run_bass_kernel_spmd(
        nc, in_maps, core_ids=list(range(NCORES)), trace=trace
    )


def kernel(q, k, v, W_out, b_out, _trace=False, _return_res=False):
    q = np.asarray(q, np.float32)
    k = np.asarray(k, np.float32)
    v = np.asarray(v, np.float32)
    W_out = np.asarray(W_out, np.float32)
    b_out = np.asarray(b_out, np.float32)

    in_maps = _prep_inputs(q, k, v, W_out)
    res = _run(in_maps, trace=_trace)

    y = np.empty((B, T, C), np.float32)
    for b in range(B):
        y[b] = res.results[2 * b]["yp"].astype(np.float32) + res.results[2 * b + 1][
            "yp"
        ].astype(np.float32)
    y += b_out[None, None, :]
    if _return_res:
        return y, res
    return y



# revision 21
# speedup vs baseline: 1.1171x; 1.1171x over previous
"""Trainium2 Bass kernel for causal multi-head attention + output projection.

Problem: B=4, T=2048, C=1024, H=16 heads (hd=64), causal softmax with
scale C**-0.5, then nn.Linear(C, C): y = attn_out @ W_out.T + b_out.

Sharding (8 cores): core = (b, head_half); b = core // 2, half = core % 2.
Each core computes 8 heads (4 head-pairs) over ALL 2048 rows of its batch
element -- every core runs the identical SPMD program (the causal loop
structure does not depend on the core id; only the data differs).  The
output projection contracts only the core's 512 channels, producing a
partial sum; the host adds the two partials per batch (replacing the
all-reduce) and adds the bias.

On-chip layout notes:
 - scoresT orientation: scores^T[k, q] = kT.T @ qT per head, so softmax
   denominators come from a ones-column appended to V (attn@V computes
   [65, q]: rows 0..63 = head dims, row 64 = sum of exp).
 - q/k/W are pre-transposed on the host (bf16), so no on-chip transposes.
 - Head pairs run as concurrent K=64 row-tiled matmuls (partitions 0-63 /
   64-127 of the PE array).
 - exp runs on ACT from PSUM with scale=C**-0.5 folded in; causal masking
   multiplies a precomputed 128x128 staircase on the diagonal blocks only.
"""

import os
import sys

for _p in ("/opt/trn_rl_repo", "/root/.axon_site/_ro/trn_rl_repo"):
    if os.path.isdir(_p) and _p not in sys.path:
        sys.path.insert(0, _p)

import numpy as np
import ml_dtypes
from contextlib import ExitStack

bf16 = ml_dtypes.bfloat16

B, T, C, H, HD = 4, 2048, 1024, 16, 64
NCORES = 8
HPC = H // 2          # heads per core = 8
PAIRS = HPC // 2      # head pairs per core = 4
CH = C // 2           # channels per core = 512
SCALE = float(C) ** -0.5

_CACHED = {}

# Tuning knobs (overridable before _get_program() for sim sweeps)
TUNE = {
    "at_bufs": 6,       # attnT SBUF pool bufs
    "sc_bufs": 2,       # scores PSUM pool bufs (tiles of 2 banks each)
    "ac_bufs": 2,       # accumulator PSUM pool bufs (1 bank each)
    "pj_bufs": 2,       # projection PSUM pool bufs (1 bank each)
    "rb_in_pj": True,   # norm_b broadcast tiles from pj pool (not sc pool)
    "y_bufs": 3,
    "rb_psum_mult": True,
    "pipe_depth": 2,    # tiles attn@V trails the scores/exp stream by
    "nb_delay": 4,      # tiles between a pair's reciprocals and its norm_b
    "pp_every": 4,      # emit a deferred projection chunk every N tiles
    "pj_pool": True,
    "flush_ge": 1,
    "loop_n": 1,        # >1: repeat body in one NEFF (benchmark mode)
    "level": 4,         # build truncation for phase benchmarks (4=full)
    "proj_interleave": True,
    "proj_dma": True,
    "proj_src_const": False,
    "y_on_act": False,  # projection PSUM->SBUF copies on ACT instead of DVE
    "sc_fp8": True,     # QK^T via fp8e4m3 DoubleRow (0.5 cyc/row)
    "batch_recip": True,  # batch denominators: 1 Ln + 1 Exp per q-tile
}


def _patch_act_tables():
    # The act-table placement pass maps each ACT func to the lowest-id set
    # containing it, which thrashes between exp_and_others and natural_log
    # (2.7us per reload).  Strip Exp/Ln/Copy from every set except the
    # combined natural_log_exp_and_others so all our ACT work shares one
    # table load.  Set ids (dict order) are preserved.
    from concourse import bacc as _bacc
    from concourse.hw_specs import get_activation_tables as _orig

    if getattr(_bacc, "_mha_act_patch", False):
        return
    import concourse.mybir as mybir

    keep = {
        mybir.ActivationFunctionType.Exp,
        mybir.ActivationFunctionType.Ln,
        mybir.ActivationFunctionType.Copy,
    }

    def patched(arch):
        tables = {k: set(v) for k, v in _orig(arch).items()}
        if "natural_log_exp_and_others" in tables and keep <= tables[
            "natural_log_exp_and_others"
        ]:
            for name, fns in tables.items():
                if name != "natural_log_exp_and_others":
                    fns -= keep
        return tables

    _bacc.get_activation_tables = patched
    _bacc._mha_act_patch = True


def _build_program():
    import concourse.bass as bass
    import concourse.tile as tile
    import concourse.mybir as mybir
    from concourse import bacc

    _patch_act_tables()

    f32 = mybir.dt.float32
    bf = mybir.dt.bfloat16
    Exp = mybir.ActivationFunctionType.Exp
    Ln = mybir.ActivationFunctionType.Ln

    f8 = mybir.dt.float8e4

    nc = bacc.Bacc()
    if TUNE["sc_fp8"]:
        q8_d = nc.declare_dram_parameter("q8", [64, 2 * PAIRS * T], f8, isOutput=False)
        k8_d = nc.declare_dram_parameter("k8", [64, 2 * PAIRS * T], f8, isOutput=False)
    else:
        qT_d = nc.declare_dram_parameter("qT", [CH, T], bf, isOutput=False)
        kT_d = nc.declare_dram_parameter("kT", [CH, T], bf, isOutput=False)
    vx_d = nc.declare_dram_parameter("vx", [T, HPC * 65], bf, isOutput=False)
    wT_d = nc.declare_dram_parameter("wT", [CH, C], bf, isOutput=False)
    mk_d = nc.declare_dram_parameter("mk", [128, 256], bf, isOutput=False)
    on_d = nc.declare_dram_parameter("on", [1, 64], bf, isOutput=False)
    yp_d = nc.declare_dram_parameter("yp", [T, C], bf, isOutput=True)

    with tile.TileContext(nc) as tc, ExitStack() as ctx:
        const = ctx.enter_context(tc.tile_pool(name="const", bufs=1))

        # Persistent SBUF tensors
        if TUNE["sc_fp8"]:
            # [p, i, j*T+t]: head = 2j + p//32, channel = i*32 + p%32
            q8_sb = const.tile([64, 2, PAIRS * T], f8)
            k8_sb = const.tile([64, 2, PAIRS * T], f8)
        else:
            qT_sb = const.tile([128, PAIRS, T], bf)  # [p, j, t]; p = pair channel
            kT_sb = const.tile([128, PAIRS, T], bf)
        v_sb = const.tile([128, 16, HPC * 65], bf)   # [p, kb, h*65+e]
        wt_sb = const.tile([128, PAIRS, C], bf)      # [p, ci, n]
        mk_sb = const.tile([128, 256], bf)
        on_sb = const.tile([128, 64], bf)            # row 64 holds ones
        aoT_sb = const.tile([128, PAIRS, T], bf)     # attn outputs, transposed

        if TUNE["sc_fp8"]:
            nc.sync.dma_start(
                q8_sb[:], q8_d[:].rearrange("p (i m) -> p i m", i=2)
            )
            nc.sync.dma_start(
                k8_sb[:], k8_d[:].rearrange("p (i m) -> p i m", i=2)
            )
        for j in range(PAIRS):
            if not TUNE["sc_fp8"]:
                nc.sync.dma_start(
                    qT_sb[:, j, :],
                    qT_d[:].rearrange("(j p) t -> j p t", p=128)[j],
                )
                nc.sync.dma_start(
                    kT_sb[:, j, :],
                    kT_d[:].rearrange("(j p) t -> j p t", p=128)[j],
                )
            nc.sync.dma_start(
                wt_sb[:, j, :],
                wT_d[:].rearrange("(j p) n -> j p n", p=128)[j],
            )
        vx_r = vx_d[:].rearrange("(g kb p) e -> g p kb e", p=128, g=4)
        for g in range(4):
            nc.sync.dma_start(v_sb[:, 4 * g : 4 * (g + 1), :], vx_r[g])
        nc.sync.dma_start(mk_sb[:], mk_d[:])
        nc.sync.dma_start(on_sb[64:65, :], on_d[:])
        if TUNE["batch_recip"]:
            nc.sync.dma_start(on_sb[0:1, :], on_d[:])

        mk_v = mk_sb[:].rearrange("p (g o) -> p g o", g=2)

        atp = ctx.enter_context(tc.tile_pool(name="attnT", bufs=TUNE["at_bufs"]))
        sums = ctx.enter_context(tc.tile_pool(name="sums", bufs=3))
        rbsp = ctx.enter_context(tc.tile_pool(name="rbs", bufs=3))
        tmpp = ctx.enter_context(tc.tile_pool(name="tmp", bufs=3))
        ypool = ctx.enter_context(tc.tile_pool(name="yout", bufs=TUNE["y_bufs"]))
        ps_sc = ctx.enter_context(
            tc.tile_pool(name="ps_sc", bufs=TUNE["sc_bufs"], space="PSUM")
        )
        ps_ac = ctx.enter_context(
            tc.tile_pool(name="ps_ac", bufs=TUNE["ac_bufs"], space="PSUM")
        )
        ps_pj = (
            ctx.enter_context(
                tc.tile_pool(name="ps_pj", bufs=TUNE["pj_bufs"], space="PSUM")
            )
            if TUNE["pj_pool"]
            else None
        )

        nb_done = {0: 0, 1: 0, 2: 0, 3: 0}  # norm_b count per q-tile

        def norm_b(p):
            # deferred normalize: recips via exp(-ln), broadcast across
            # partitions (K=1 matmuls), scale the SBUF copies into aoT_sb
            jj, qq0, un, r32, pid, batched = p
            nb_done[qq0 // 512] += 1
            with nc.named_scope("norm"):
                if batched:
                    rbf, onrow = r32, on_sb[0:1, :]  # rbf_j [1, 1024] at partition 0
                else:
                    rbf = sums.tile([65, 1024], bf, tag="rbf", name=f"rbf_{pid}")
                    nc.scalar.activation(
                        out=rbf[64:65, :], in_=r32[64:65, :], func=Exp, scale=-1.0
                    )
                    rbf, onrow = rbf[64:65, :], on_sb[64:65, :]
                if TUNE["rb_in_pj"]:
                    # two 1-bank pieces from the pj pool so the sc double-
                    # buffer is never displaced by normalize work
                    rbs = [
                        ps_pj.tile([128, 512], f32, tag="pj", name=f"rb0_{pid}"),
                        ps_pj.tile([128, 512], f32, tag="pj", name=f"rb1_{pid}"),
                    ]
                    rbv = [rbs[0][0:64, :], rbs[1][0:64, :]]
                else:
                    rb = ps_sc.tile([128, 2, 512], f32, tag="sc", name=f"rb_{pid}")
                    rbv = [rb[0:64, 0, :], rb[0:64, 1, :]]
                for h in range(2):
                    nc.tensor.matmul(
                        out=rbv[h],
                        lhsT=onrow,
                        rhs=rbf[:, h * 512 : (h + 1) * 512],
                        start=True,
                        stop=True,
                    )
                rb0, rb1 = rbv
                nc.vector.tensor_mul(
                    out=aoT_sb[0:64, jj, qq0 : qq0 + 512],
                    in0=un[0:64, 0:512],
                    in1=rb0,
                )
                t1 = tmpp.tile([64, 512], bf, tag="t1", name=f"t1_{pid}")
                nc.vector.tensor_mul(out=t1[:, :], in0=un[0:64, 512:1024], in1=rb1)
                nc.sync.dma_start(aoT_sb[64:128, jj, qq0 : qq0 + 512], t1[:, :])

        def emit_proj(qc):
            # partial projection (contract the core's CH channels) for one
            # 128-row chunk
            if LEVEL < 4:
                return
            src_t = wt_sb if TUNE["proj_src_const"] else aoT_sb
            with nc.named_scope("proj"):
                y_sb = ypool.tile([128, C], bf, tag="y", name=f"y_{qc}")
                pjt = (
                    None
                    if ps_pj is not None
                    else ps_sc.tile([128, 2, 512], f32, tag="sc", name=f"pj_{qc}")
                )
                for nt in range(2):
                    if pjt is None:
                        pj = ps_pj.tile([128, 512], f32, tag="pj", name=f"pj_{qc}_{nt}")
                    else:
                        pj = pjt[:, nt, :]
                    for ci in range(PAIRS):
                        nc.tensor.matmul(
                            out=pj[:, :],
                            lhsT=src_t[:, ci, qc * 128 % src_t.shape[2] : qc * 128 % src_t.shape[2] + 128],
                            rhs=wt_sb[:, ci, nt * 512 : (nt + 1) * 512],
                            start=(ci == 0),
                            stop=(ci == PAIRS - 1),
                        )
                    if TUNE["y_on_act"]:
                        nc.scalar.copy(y_sb[:, nt * 512 : (nt + 1) * 512], pj[:, :])
                    else:
                        nc.vector.tensor_copy(
                            y_sb[:, nt * 512 : (nt + 1) * 512], pj[:, :]
                        )
                if TUNE["proj_dma"]:
                    nc.sync.dma_start(yp_d[qc * 128 : (qc + 1) * 128, :], y_sb[:, :])

        pending_proj = []

        # Optional benchmark loop: repeat the whole compute body loop_n
        # times in one NEFF (the body is idempotent) to measure HW time as
        # a wall-clock delta without per-call transfer overhead.
        loop_ctx = (
            tc.For_i(0, TUNE["loop_n"], 1) if TUNE["loop_n"] > 1 else None
        )
        if loop_ctx is not None:
            ctx.enter_context(loop_ctx)

        # Flat software-pipelined stream over (pair, kb): scores+exp lead,
        # attn@V trails by pipe_depth tiles (crossing pair boundaries), the
        # normalize broadcast trails one pair, projection trails one q-tile.
        DEPTH = TUNE["pipe_depth"]
        LEVEL = TUNE["level"]
        NB_DELAY = TUNE["nb_delay"]
        PP_EVERY = TUNE["pp_every"]
        normed = []  # (tick, norm_b payload) awaiting emission
        tick = [0]
        accs = {}        # (qt, j) -> [acc0, acc1]
        pends = []       # (qt, j, kb, c0, at, nkb)
        done_pairs = []  # (qt, j) pairs whose last attn@V has been emitted
        staged = {}      # qt -> [(j, un), ...] awaiting the batched Ln/Exp
        srows = {}       # qt -> srow tile

        def pop_av():
            pqt, pj_, pkb, pc0, pat, pnkb = pends.pop(0)
            if pkb == 0 and not TUNE["batch_recip"]:
                # free the oldest pair's acc banks before a new pair's
                # accumulation claims slots (deadlock avoidance): with 4
                # acc bufs, two pairs may be in flight, so only the
                # second-oldest pending normalize must flush here
                while len(normed) >= TUNE["flush_ge"]:
                    norm_b(normed.pop(0)[1])
            pacc = accs[(pqt, pj_)]
            if LEVEL >= 2:
                with nc.named_scope("av"):
                    for h in range(2):
                        nc.tensor.matmul(
                            out=pacc[h][0:65, pc0:],
                            lhsT=v_sb[
                                :, pkb, (2 * pj_ + h) * 65 : (2 * pj_ + h) * 65 + 65
                            ],
                            rhs=pat[:, h, pc0:],
                            start=(pkb == 0),
                            stop=(pkb == pnkb - 1),
                        )
            if pkb == pnkb - 1:
                done_pairs.append((pqt, pj_))

        def emit_batch(dqt):
            # one Ln + one Exp for all 4 pairs' denominators, then peel
            # each pair's reciprocal row back to partition 0 for the
            # K=1 broadcast matmul in norm_b
            with nc.named_scope("norm"):
                srow_t = srows.pop(dqt)
                r32b = sums.tile(
                    [4, 1024], f32, tag="r32b", bufs=2, name=f"r32b_{dqt}"
                )
                nc.scalar.activation(out=r32b[:, :], in_=srow_t[:, :], func=Ln)
                rbfb = sums.tile(
                    [4, 1024], bf, tag="rbfb", bufs=2, name=f"rbfb_{dqt}"
                )
                nc.scalar.activation(
                    out=rbfb[:, :], in_=r32b[:, :], func=Exp, scale=-1.0
                )
                for dj, un in staged.pop(dqt):
                    rbfj = sums.tile(
                        [1, 1024], bf, tag="rbfj", bufs=8, name=f"rbfj_{dqt}_{dj}"
                    )
                    nc.sync.dma_start(rbfj[:, :], rbfb[dj : dj + 1, :])
                    normed.append(
                        (tick[0], (dj, dqt * 512, un, rbfj, f"{dqt}_{dj}", True))
                    )

        def flush_done():
            # norm_a (reciprocals) for any pair whose attn@V finished
            while done_pairs:
                dqt, dj = done_pairs.pop(0)
                dacc = accs.pop((dqt, dj))
                if LEVEL < 3:
                    continue
                # qt<3 batches all 4 pairs' reciprocals into one Ln+Exp;
                # qt=3 keeps the per-pair path so only the last pair's
                # normalize chain sits in the kernel tail.
                use_batch = TUNE["batch_recip"] and dqt < 3
                # Release the acc PSUM banks fast: DVE copies of the
                # unnormalized outputs (+sum row) to SBUF.
                with nc.named_scope("norm"):
                    if use_batch:
                        if dqt not in srows:
                            srows[dqt] = sums.tile(
                                [4, 1024], f32, tag="srow", bufs=2,
                                name=f"srow_{dqt}",
                            )
                        un = sums.tile(
                            [65, 1024], f32, tag="un", bufs=6,
                            name=f"un_{dqt}_{dj}",
                        )
                        nc.vector.tensor_copy(un[0:65, 0:512], dacc[0][0:65, :])
                        nc.vector.tensor_copy(un[0:65, 512:1024], dacc[1][0:65, :])
                        nc.sync.dma_start(
                            srows[dqt][dj : dj + 1, 0:512], un[64:65, 0:512]
                        )
                        nc.sync.dma_start(
                            srows[dqt][dj : dj + 1, 512:1024], un[64:65, 512:1024]
                        )
                    else:
                        r32 = sums.tile(
                            [65, 1024], f32, tag="r32", name=f"r32_{dqt}_{dj}"
                        )
                        nc.scalar.activation(
                            out=r32[64:65, 0:512], in_=dacc[0][64:65, :], func=Ln
                        )
                        nc.scalar.activation(
                            out=r32[64:65, 512:1024], in_=dacc[1][64:65, :], func=Ln
                        )
                        un = sums.tile(
                            [65, 1024], f32, tag="un", bufs=6,
                            name=f"un_{dqt}_{dj}",
                        )
                        nc.vector.tensor_copy(un[0:64, 0:512], dacc[0][0:64, :])
                        nc.vector.tensor_copy(un[0:64, 512:1024], dacc[1][0:64, :])
                if use_batch:
                    staged.setdefault(dqt, []).append((dj, un))
                    if len(staged[dqt]) == PAIRS:
                        emit_batch(dqt)
                else:
                    normed.append(
                        (tick[0], (dj, dqt * 512, un, r32, f"{dqt}_{dj}", False))
                    )

        for qt in range(4):
            q0 = qt * 512
            nkb = 4 * qt + 4
            for j in range(PAIRS):
                accs[(qt, j)] = [
                    ps_ac.tile([128, 512], f32, tag="acc", name=f"acc0_{qt}_{j}"),
                    ps_ac.tile([128, 512], f32, tag="acc", name=f"acc1_{qt}_{j}"),
                ]
                for kb in range(nkb):
                    c0 = max(0, (kb - 4 * qt) * 128)
                    sc = ps_sc.tile(
                        [128, 2, 512], f32, tag="sc", name=f"sc_{qt}_{j}_{kb}"
                    )
                    with nc.named_scope("sc"):
                        for h in range(2):
                            if TUNE["sc_fp8"]:
                                # DoubleRow: both 32-channel halves of the
                                # head contract in one 0.5-cyc/row pass
                                nc.tensor.matmul(
                                    out=sc[:, h, c0:],
                                    lhsT=k8_sb[
                                        h * 32 : (h + 1) * 32,
                                        :,
                                        j * T + kb * 128 : j * T + (kb + 1) * 128,
                                    ],
                                    rhs=q8_sb[
                                        h * 32 : (h + 1) * 32,
                                        :,
                                        j * T + q0 + c0 : j * T + q0 + 512,
                                    ],
                                    start=True,
                                    stop=True,
                                    perf_mode=mybir.MatmulPerfMode.DoubleRow,
                                )
                            else:
                                nc.tensor.matmul(
                                    out=sc[:, h, c0:],
                                    lhsT=kT_sb[
                                        h * 64 : (h + 1) * 64,
                                        j,
                                        kb * 128 : (kb + 1) * 128,
                                    ],
                                    rhs=qT_sb[
                                        h * 64 : (h + 1) * 64, j, q0 + c0 : q0 + 512
                                    ],
                                    start=True,
                                    stop=True,
                                )
                    at = atp.tile([128, 2, 512], bf, tag="at", name=f"at_{qt}_{j}_{kb}")
                    if LEVEL >= 1:
                        with nc.named_scope("exp"):
                            nc.scalar.activation(
                                out=at[:, :, c0:],
                                in_=sc[:, :, c0:],
                                func=Exp,
                                scale=SCALE,
                            )
                        if kb >= 4 * qt:  # diagonal block: causal staircase mask
                            with nc.named_scope("mask"):
                                nc.vector.tensor_mul(
                                    out=at[:, :, c0 : c0 + 128],
                                    in0=at[:, :, c0 : c0 + 128],
                                    in1=mk_v,
                                )
                    pends.append((qt, j, kb, c0, at, nkb))
                    if len(pends) > DEPTH:
                        pop_av()
                        flush_done()
                    tick[0] += 1
                    # interleave deferred work from previous pairs/q-tiles
                    if normed and tick[0] - normed[0][0] >= NB_DELAY:
                        norm_b(normed.pop(0)[1])
                    if (
                        TUNE["proj_interleave"]
                        and pending_proj
                        and tick[0] % PP_EVERY == 0
                        and nb_done[pending_proj[0] // 4] == PAIRS
                    ):
                        emit_proj(pending_proj.pop(0))

            pending_proj.extend(range(4 * qt, 4 * qt + 4))

        # endgame: drain the trailing attn@V / normalize / projection work
        while pends:
            pop_av()
            flush_done()
        flush_done()
        while normed:
            norm_b(normed.pop(0)[1])
        while pending_proj:
            emit_proj(pending_proj.pop(0))

    nc.finalize()
    return nc


def _get_program():
    if "nc" not in _CACHED:
        _CACHED["nc"] = _build_program()
    return _CACHED["nc"]


def _prep_inputs(q, k, v, W_out):
    """Build the 8 per-core input maps (host-side shard + transpose + cast)."""
    mk = np.zeros((128, 2, 128), np.float32)
    kk = np.arange(128)[:, None]
    oo = np.arange(128)[None, :]
    mk[:, 0, :] = (kk <= oo).astype(np.float32)
    mk[:, 1, :] = mk[:, 0, :]
    mk = mk.reshape(128, 256).astype(bf16)
    ones = np.ones((1, 64), bf16)

    f8np = ml_dtypes.float8_e4m3

    def pack8(x, ch0):
        # [p, i, j*T+t]: head 2j + p//32, channel i*32 + p%32 (within head)
        xc = np.ascontiguousarray(x.T[ch0 : ch0 + CH])  # [512 ch, T]
        xc = xc.reshape(PAIRS, 2, 2, 32, T)  # [j, hb, i, p32, t]
        xp = xc.transpose(1, 3, 2, 0, 4).reshape(64, 2 * PAIRS * T)
        return np.ascontiguousarray(xp).astype(f8np)

    in_maps = []
    for core in range(NCORES):
        b, hh = core // 2, core % 2
        ch0 = hh * CH
        vh = v[b].reshape(T, H, HD)[:, hh * HPC : (hh + 1) * HPC, :]
        vx = np.concatenate(
            [vh.astype(np.float32), np.ones((T, HPC, 1), np.float32)], axis=2
        )
        vx = np.ascontiguousarray(vx.reshape(T, HPC * 65)).astype(bf16)
        wT = np.ascontiguousarray(W_out.T[ch0 : ch0 + CH]).astype(bf16)
        m = {"vx": vx, "wT": wT, "mk": mk, "on": ones}
        if TUNE["sc_fp8"]:
            m["q8"] = pack8(q[b], ch0)
            m["k8"] = pack8(k[b], ch0)
        else:
            m["qT"] = np.ascontiguousarray(q[b].T[ch0 : ch0 + CH]).astype(bf16)
            m["kT"] = np.ascontiguousarray(k[b].T[ch0 : ch0 + CH]).astype(bf16)
        in_maps.append(m)
    return in_maps


def _run(in_maps, trace=False):
    from concourse.bass_utils import run_bass_kernel_spmd

    nc = _get_program()
    return run_bass_kernel_spmd(
        nc, in_maps, core_ids=list(range(NCORES)), trace=trace
    )


def kernel(q, k, v, W_out, b_out, _trace=False, _return_res=False):
    q = np.asarray(q, np.float32)
    k = np.asarray(k, np.float32)
    v = np.asarray(v, np.float32)
    W_out = np.asarray(W_out, np.float32)
    b_out = np.asarray(b_out, np.float32)

    in_maps = _prep_inputs(q, k, v, W_out)
    res = _run(in_maps, trace=_trace)

    y = np.empty((B, T, C), np.float32)
    for b in range(B):
        y[b] = res.results[2 * b]["yp"].astype(np.float32) + res.results[2 * b + 1][
            "yp"
        ].astype(np.float32)
    y += b_out[None, None, :]
    if _return_res:
        return y, res
    return y

